# revision 18
# baseline (speedup 1.0000x reference)
"""GCN layer (message passing + linear + BatchNorm + ReLU + residual) on 8 TRN2 cores.

Strategy (graph/data parallel, per sharding hint):
  - Nodes (and their incident edges, grouped by dst) are sharded across 8 cores.
  - Feature rows are gathered from a replicated DRAM copy via SWDGE dma_gather
    in float8e3 (e3m4): 128B descriptors (one row each) out of a 256B-stride
    pair table so idx fits int16 (idx = src>>1, even/odd views pick parity).
    Multi-packet gathers batch up to 8192 idx per call (HW-verified), cutting
    SWDGE desc-gen overhead ~10x vs 1024-idx calls.
  - Per 128-node dst tile: segment-sum is a one-hot(dst) matmul (e3m4 msgs x
    bf16 one-hot) accumulating in PSUM; mean = multiply by 1/deg broadcast
    (PE outer-product in bf16 hi/lo); linear = W @ h1 with W^T in bf16 hi/lo.
  - One-hot builds are split DVE/Pool to balance engine occupancy.
  - BatchNorm batch stats are all-reduced across cores ([128,2] AllReduce).
  - b is omitted on device: training-mode BN makes the output invariant to b.
  - Residual feature and output travel as bf16 (rel tolerance 2e-2; measured
    end-to-end error ~1.2e-2 dominated by e3m4 message rounding).
"""

import sys

for _p in ("/opt/trn_rl_repo", "/root/.axon_site/_ro/trn_rl_repo"):
    if _p not in sys.path:
        sys.path.insert(0, _p)

import numpy as np
from contextlib import ExitStack

import concourse.bass as bass
import concourse.bacc as bacc
import concourse.mybir as mybir
import concourse.tile as tile
from concourse.bass_utils import run_bass_kernel_spmd

try:
    from ml_dtypes import bfloat16 as np_bf16, float8_e3m4 as np_e3m4
except ImportError:  # jax always ships ml_dtypes
    import jax.numpy as _jnp

    np_bf16 = _jnp.bfloat16
    np_e3m4 = _jnp.float8_e3m4

F32 = mybir.dt.float32
BF16 = mybir.dt.bfloat16
E3M4 = mybir.dt.float8e3
I16 = mybir.dt.int16

P = 128          # partitions / tile node count / edge block size
D = 128          # feature dim
NCORES = 8
BN_EPS = 1e-5
GMAX = 8192      # max idx per dma_gather call (multi-packet; HW-bisected)
POOL_OH_MOD = 10**9  # Pool one-hots stall PE behind gather desc-gen; keep all on DVE


# ---------------------------------------------------------------- host prep

def _split_hi_lo(x32):
    """f32 -> (hi, lo) bf16 pair with hi+lo ~= x to ~2^-16 relative."""
    hi = x32.astype(np_bf16)
    lo = (x32 - hi.astype(np.float32)).astype(np_bf16)
    return hi, lo


def host_prep(feature, W, gamma, beta, src, dst, ncores=NCORES):
    """Index-only graph preprocessing + data layout. Returns (cfg, in_maps)."""
    n, d = feature.shape
    assert d == D
    shard = -(-n // (ncores * P)) * P          # per-core node count, mult of 128
    npad = shard * ncores
    nt = shard // P                            # dst tiles per core

    src = np.asarray(src, dtype=np.int64)
    dst = np.asarray(dst, dtype=np.int64)

    # degree / reciprocal (index-only; 0 where deg==0 implements the where())
    deg = np.bincount(dst, minlength=npad).astype(np.float64)
    recip = np.where(deg > 0, 1.0 / np.maximum(deg, 1.0), 0.0).astype(np.float32)

    # sort edges by destination; per (core, tile, parity-of-src) edge lists
    order = np.argsort(dst, kind="stable")
    s_src = src[order]
    s_dst = dst[order]
    tile_of = (s_dst // P).astype(np.int64)
    ntiles_g = ncores * nt
    bounds = np.searchsorted(tile_of, np.arange(ntiles_g + 1))

    lists = []  # global tile -> (ev_src, od_src, ev_dst, od_dst)
    cnt = np.zeros((ncores, nt, 2), dtype=np.int64)
    for tg in range(ntiles_g):
        lo_, hi_ = bounds[tg], bounds[tg + 1]
        ts_, td_ = s_src[lo_:hi_], s_dst[lo_:hi_]
        par = (ts_ & 1).astype(bool)
        c, tl = divmod(tg, nt)
        cnt[c, tl, 0] = int((~par).sum())
        cnt[c, tl, 1] = int(par.sum())
        lists.append((ts_[~par], ts_[par], td_[~par], td_[par]))

    # uniform (across cores) per-(tile,parity) block caps
    nblk_tp = np.zeros((nt, 2), dtype=np.int64)   # blocks of 128
    for t in range(nt):
        for p in (0, 1):
            nblk_tp[t, p] = max(1, int(-(-cnt[:, t, p].max() // P)))
    nidx_tp = nblk_tp * P

    # greedy tile groups: per group, sum of ev idx <= GMAX and od idx <= GMAX;
    # the last group is split so the final aggregation tail is short
    groups = []
    cur, ev_sum, od_sum = [], 0, 0
    for t in range(nt):
        e_, o_ = int(nidx_tp[t, 0]), int(nidx_tp[t, 1])
        if cur and (ev_sum + e_ > GMAX or od_sum + o_ > GMAX):
            groups.append(cur)
            cur, ev_sum, od_sum = [], 0, 0
        cur.append(t)
        ev_sum += e_
        od_sum += o_
    if cur:
        groups.append(cur)
    if len(groups[-1]) > 4:
        last = groups.pop()
        groups.append(last[:4])
        groups.append(last[4:])

    grp_e = [int(sum(nidx_tp[t, 0] for t in g)) for g in groups]
    grp_o = [int(sum(nidx_tp[t, 1] for t in g)) for g in groups]
    gbmax = max((e_ + o_) // P for e_, o_ in zip(grp_e, grp_o))
    total_idx = int(nidx_tp.sum())
    total_blk = total_idx // P
    icols = total_idx // 16

    # e3m4 pair table: row r = [x_{2r} | x_{2r+1}], 256B stride
    xpad = np.zeros((npad, D), dtype=np.float32)
    xpad[:n] = np.asarray(feature, dtype=np.float32)
    x8 = xpad.astype(np_e3m4)
    xpair = np.ascontiguousarray(x8.reshape(npad // 2, 2 * D))

    # W^T hi/lo (lhsT layout [in_feat, out_feat])
    wt = np.ascontiguousarray(np.asarray(W, dtype=np.float32).T)
    wthi, wtlo = _split_hi_lo(wt)
    wtcat = np.concatenate([wthi, wtlo], axis=1)             # [128, 256] bf16

    iotab = np.ascontiguousarray(
        np.broadcast_to(np.arange(P, dtype=np.float32), (P, P))
    ).astype(np_bf16)

    gb = np.stack(
        [np.asarray(gamma, np.float32), np.asarray(beta, np.float32)], axis=1
    )  # [128, 2]

    in_maps = []
    for c in range(ncores):
        idx_cols = np.zeros((16, icols), dtype=np.int16)
        dl = np.full((P, total_blk), -1.0, dtype=np.float32)
        rdl = np.zeros((P, total_blk), dtype=np.float32)
        io = 0
        bo = 0
        # stream order: per group, [ev segs of tiles | od segs of tiles]
        for g in groups:
            for pa in (0, 1):
                for t in g:
                    ev_s, od_s, ev_d, od_d = lists[c * nt + t]
                    ss, dd = (ev_s, ev_d) if pa == 0 else (od_s, od_d)
                    cap = int(nidx_tp[t, pa])
                    vals = np.zeros(cap, dtype=np.int16)
                    vals[: ss.shape[0]] = (ss >> 1).astype(np.int16)
                    idx_cols[:, io : io + cap // 16] = vals.reshape(cap // 16, 16).T
                    io += cap // 16
                    dloc = np.full(cap, -1.0, dtype=np.float32)
                    dloc[: dd.shape[0]] = (dd % P).astype(np.float32)
                    dl[:, bo : bo + cap // P] = dloc.reshape(cap // P, P).T
                    rloc = np.zeros(cap, dtype=np.float32)
                    rloc[: dd.shape[0]] = recip[dd]
                    rdl[:, bo : bo + cap // P] = rloc.reshape(cap // P, P).T
                    bo += cap // P
        assert io == icols and bo == total_blk
        idx_rep = np.tile(idx_cols, (8, 1))                   # replicate to 128 parts

        # residual feature transposed, bf16
        xres = np.ascontiguousarray(
            xpad[c * shard : (c + 1) * shard].T.astype(np_bf16)
        )

        in_maps.append(
            {
                "xpair": xpair,
                "xres": xres,
                "idx": np.ascontiguousarray(idx_rep),
                "dstloc": np.ascontiguousarray(dl),
                "recdl": np.ascontiguousarray(rdl),
                "wt": np.ascontiguousarray(wtcat),
                "iotab": iotab,
                "gb": np.ascontiguousarray(gb),
            }
        )

    cfg = {
        "n": n,
        "npad": npad,
        "shard": shard,
        "nt": nt,
        "nblk_tp": nblk_tp,
        "nidx_tp": nidx_tp,
        "groups": groups,
        "grp_e": grp_e,
        "grp_o": grp_o,
        "gbmax": gbmax,
        "total_blk": total_blk,
        "icols": icols,
        "ncores": ncores,
    }
    return cfg, in_maps


# ---------------------------------------------------------------- device program

def dma_gather_raw(eng, out_ap, in_ap, idxs_ap, num_idxs, elem_size, elem_step):
    """bass.dma_gather without the elem%256B restriction (128B descriptors
    HW-verified correct in non-transpose mode; stride must stay 256B-aligned)."""
    from concourse.bass import exact_div

    stride_bytes_256 = exact_div(elem_step * mybir.dt.size(in_ap.dtype), 256)
    _in_ap = eng.lower_ap_dma(in_ap, for_custom_bir_dma=True)
    return eng.add_instruction(
        mybir.InstDMAGatherAnt(
            name=eng.bass.get_next_instruction_name(),
            ins=[*_in_ap, eng.lower_ap(idxs_ap),
                 eng.lower_val_access(eng.to_reg(num_idxs))],
            outs=[eng.lower_ap(out_ap)],
            transpose=False,
            num_idxs=num_idxs,
            elem_size=elem_size,
            stride_bytes_256=stride_bytes_256,
            gen_mode=0,
            single_packet=False,
            queue_num=0,
            sbuf_tokens_per_rank=0,
            sbuf_free_dim_per_rank=0,
            sbuf_free_dim_pad_per_rank=0,
            sbuf_byte_offset=0,
        )
    )


def build_program(cfg, skip_collective=False):
    ncores = cfg["ncores"]
    shard, nt = cfg["shard"], cfg["nt"]
    npad = cfg["npad"]
    nblk_tp, nidx_tp = cfg["nblk_tp"], cfg["nidx_tp"]
    groups, grp_e, grp_o = cfg["groups"], cfg["grp_e"], cfg["grp_o"]
    gbmax = cfg["gbmax"]
    total_blk, icols = cfg["total_blk"], cfg["icols"]
    inv_n = 1.0 / cfg["n"]

    nc = bacc.Bacc("TRN2", target_bir_lowering=False, debug=False,
                   num_devices=ncores)

    xpair_d = nc.declare_dram_parameter("xpair", [npad // 2, 2 * D], E3M4, False)
    xres_d = nc.declare_dram_parameter("xres", [P, shard], BF16, False)
    idx_d = nc.declare_dram_parameter("idx", [P, icols], I16, False)
    dl_d = nc.declare_dram_parameter("dstloc", [P, total_blk], F32, False)
    rdl_d = nc.declare_dram_parameter("recdl", [P, total_blk], F32, False)
    wt_d = nc.declare_dram_parameter("wt", [P, 2 * D], BF16, False)
    io_d = nc.declare_dram_parameter("iotab", [P, P], BF16, False)
    gb_d = nc.declare_dram_parameter("gb", [P, 2], F32, False)
    out_d = nc.declare_dram_parameter("outt", [P, shard], BF16, True)

    AL = mybir.AluOpType
    AF = mybir.ActivationFunctionType

    with ExitStack() as ctx:
        tc = ctx.enter_context(tile.TileContext(nc))
        const = ctx.enter_context(tc.tile_pool(name="const", bufs=1))
        mpool = ctx.enter_context(tc.tile_pool(name="msgs", bufs=4))
        ohpool = ctx.enter_context(tc.tile_pool(name="oh", bufs=8))
        hpool = ctx.enter_context(tc.tile_pool(name="h1", bufs=3))
        scpool = ctx.enter_context(tc.tile_pool(name="scratch", bufs=2))
        xpool = ctx.enter_context(tc.tile_pool(name="xres", bufs=3))
        stat = ctx.enter_context(tc.tile_pool(name="stat", bufs=1))
        apool = ctx.enter_context(tc.tile_pool(name="aggps", bufs=2, space="PSUM"))
        zpool = ctx.enter_context(tc.tile_pool(name="zps", bufs=2, space="PSUM"))
        dram = ctx.enter_context(tc.tile_pool(name="dram", bufs=2, space="DRAM"))

        idx_s = const.tile([P, icols], I16)
        dl_s = const.tile([P, total_blk], F32)
        rdl_s = const.tile([P, total_blk], F32)
        wt_s = const.tile([P, 2 * D], BF16)
        io_s = const.tile([P, P], BF16)
        gb_s = const.tile([P, 2], F32)
        zbuf = const.tile([P, shard], BF16)
        sacc = const.tile([P, nt], F32)
        qacc = const.tile([P, nt], F32)

        nfront = min(4, len(groups))
        g0i = (grp_e[0] + grp_o[0]) // 16
        g0b = (grp_e[0] + grp_o[0]) // P
        g01i = sum((grp_e[i] + grp_o[i]) // 16 for i in range(nfront))
        g01b = sum((grp_e[i] + grp_o[i]) // P for i in range(nfront))
        nc.sync.dma_start(idx_s[:, 0:g0i], idx_d[:, 0:g0i])
        nc.sync.dma_start(dl_s[:, 0:g0b], dl_d[:, 0:g0b])
        nc.sync.dma_start(rdl_s[:, 0:g0b], rdl_d[:, 0:g0b])
        nc.sync.dma_start(io_s[:], io_d[:])
        nc.sync.dma_start(wt_s[:], wt_d[:])
        nc.sync.dma_start(gb_s[:], gb_d[:])
        nc.sync.dma_start(dl_s[:, g0b:g01b], dl_d[:, g0b:g01b])
        nc.sync.dma_start(rdl_s[:, g0b:g01b], rdl_d[:, g0b:g01b])
        nc.sync.dma_start(idx_s[:, g0i:g01i], idx_d[:, g0i:g01i])

        ev_view = xpair_d[:, 0:D]     # even rows, 128B elems at 256B stride
        od_view = xpair_d[:, D:2 * D]  # odd rows

        h1all = const.tile([P, shard], BF16)
        xall = const.tile([P, shard], BF16)
        ohwmax = int((nblk_tp[:, 0] + nblk_tp[:, 1]).max())
        tiles_aggregated = []

        def emit_tail(t):
            """W matmuls + BN-stat accumulation for an aggregated tile."""
            zp = zpool.tile([P, P], F32, tag="zp")
            h1sl = h1all[:, t * P : (t + 1) * P]
            nc.tensor.matmul(zp[:], wt_s[:, 0:D], h1sl, start=True, stop=False)
            nc.tensor.matmul(zp[:], wt_s[:, D : 2 * D], h1sl, start=False,
                             stop=True)
            nc.scalar.activation(
                zbuf[:, t * P : (t + 1) * P], zp[:], AF.Copy,
                accum_out=sacc[:, t : t + 1],
            )
            sq = scpool.tile([P, P], BF16, tag="sq")
            nc.scalar.activation(
                sq[:], zp[:], AF.Square, accum_out=qacc[:, t : t + 1]
            )

        io_off = 0
        blk_off = 0
        ohk = 0
        for gi, g in enumerate(groups):
            # NOTE: uniform shape per pool tag — differently-shaped tiles
            # sharing a tag produce a device-crashing NEFF (HW-bisected).
            msgs = mpool.tile([P, gbmax, D], E3M4, tag="msgs")
            eg, og = grp_e[gi], grp_o[gi]
            gchunk = 1024 if gi == 0 else GMAX
            for view, cnt_, blk0 in ((ev_view, eg, 0), (od_view, og, eg // P)):
                for off in range(0, cnt_, gchunk):
                    ln = min(gchunk, cnt_ - off)
                    dma_gather_raw(
                        nc.gpsimd,
                        msgs[:, blk0 + off // P : blk0 + (off + ln) // P, :],
                        view,
                        idx_s[:, io_off + off // 16 : io_off + (off + ln) // 16],
                        ln, D, 2 * D,
                    )
                io_off += cnt_ // 16

            if gi == 3:
                # bulk loads ride the Pool queue so their transfers trail the
                # first two groups' gathers in the FIFO DMA device
                nc.gpsimd.dma_start(dl_s[:, g01b:], dl_d[:, g01b:])
                nc.gpsimd.dma_start(rdl_s[:, g01b:], rdl_d[:, g01b:])
                nc.gpsimd.dma_start(idx_s[:, g01i:], idx_d[:, g01i:])
                nc.gpsimd.dma_start(xall[:], xres_d[:])

            ev_base = 0
            od_base = eg // P
            for t in g:
                nbe, nbo = int(nblk_tp[t, 0]), int(nblk_tp[t, 1])
                tile_blocks = [ev_base + b for b in range(nbe)] + [
                    od_base + b for b in range(nbo)
                ]
                ev_base += nbe
                od_base += nbo
                agg = apool.tile([P, P], F32, tag="agg")
                # all one-hots of this tile share one wide tile: buffer-reuse
                # sync happens per tile instead of per block
                ohw = ohpool.tile([P, ohwmax * P], BF16, tag="ohw")
                for k, mb in enumerate(tile_blocks):
                    ohsl = ohw[:, k * P : (k + 1) * P]
                    eng = nc.gpsimd if (ohk % POOL_OH_MOD == POOL_OH_MOD - 1) \
                        else nc.vector
                    eng.tensor_scalar(
                        ohsl, io_s[:],
                        dl_s[:, blk_off + mb : blk_off + mb + 1],
                        rdl_s[:, blk_off + mb : blk_off + mb + 1],
                        AL.is_equal, AL.mult,
                    )
                    ohk += 1
                    nc.tensor.matmul(
                        agg[:], msgs[:, mb, :], ohsl,
                        start=(k == 0), stop=(k == len(tile_blocks) - 1),
                    )

                nc.scalar.activation(
                    h1all[:, t * P : (t + 1) * P], agg[:], AF.Copy
                )
                tiles_aggregated.append(t)
                if len(tiles_aggregated) >= 3:
                    emit_tail(tiles_aggregated.pop(0))
            blk_off += (eg + og) // P

        for t in tiles_aggregated:
            emit_tail(t)

        # ---- BatchNorm statistics (global over all cores) ----
        ssum = stat.tile([P, 2], F32)
        nc.vector.tensor_reduce(
            ssum[:, 0:1], sacc[:], axis=mybir.AxisListType.X, op=AL.add
        )
        nc.vector.tensor_reduce(
            ssum[:, 1:2], qacc[:], axis=mybir.AxisListType.X, op=AL.add
        )

        tot = stat.tile([P, 2], F32)
        if skip_collective:
            nc.vector.tensor_copy(tot[:], ssum[:])
        else:
            cin = dram.tile([P, 2], F32)
            cout = dram.tile([P, 2], F32)
            nc.gpsimd.dma_start(cin[:], ssum[:])
            nc.gpsimd.collective_compute(
                "AllReduce",
                AL.add,
                replica_groups=[list(range(ncores))],
                ins=[cin.opt()],
                outs=[cout.opt()],
            )
            nc.gpsimd.dma_start(tot[:], cout[:])

        # scale = gamma / sqrt(var+eps); shift = beta - mu*scale
        mu = stat.tile([P, 1], F32)
        nc.vector.tensor_scalar(mu[:], tot[:, 0:1], inv_n, None, AL.mult)
        e2 = stat.tile([P, 1], F32)
        nc.vector.tensor_scalar(e2[:], tot[:, 1:2], inv_n, None, AL.mult)
        var = stat.tile([P, 1], F32)
        nc.vector.tensor_mul(var[:], mu[:], mu[:])
        nc.vector.tensor_sub(var[:], e2[:], var[:])
        epsb = stat.tile([P, 1], F32)
        nc.vector.memset(epsb[:], float(BN_EPS))
        sd = stat.tile([P, 1], F32)
        nc.scalar.activation(sd[:], var[:], AF.Sqrt, bias=epsb[:])
        inv = stat.tile([P, 1], F32)
        nc.vector.reciprocal(inv[:], sd[:])
        scale = stat.tile([P, 1], F32)
        nc.vector.tensor_mul(scale[:], gb_s[:, 0:1], inv[:])
        shift = stat.tile([P, 1], F32)
        nc.vector.tensor_mul(shift[:], mu[:], scale[:])
        nc.vector.tensor_sub(shift[:], gb_s[:, 1:2], shift[:])

        # ---- finalize: out = x + relu(z*scale + shift); x streamed in bf16,
        # output written per 7-tile chunk so DMA overlaps the compute ----
        FW = 7 * P
        for t in range(0, nt, 7):
            w = min(FW, (nt - t) * P)
            sl = slice(t * P, t * P + w)
            tmp = scpool.tile([P, FW], BF16, tag="fin")
            nc.scalar.activation(
                tmp[:, 0:w], zbuf[:, sl], AF.Relu, bias=shift[:], scale=scale[:]
            )
            nc.vector.tensor_add(zbuf[:, sl], tmp[:, 0:w], xall[:, sl])
            nc.sync.dma_start(out_d[:, sl], zbuf[:, sl])

    nc.compile()
    return nc


# ---------------------------------------------------------------- entry point

def kernel(feature, W, b, gamma, beta, src, dst, _trace=False,
           _skip_collective=False):
    n = feature.shape[0]
    cfg, in_maps = host_prep(feature, W, gamma, beta, src, dst)
    nc = build_program(cfg, skip_collective=_skip_collective)
    try:
        res = run_bass_kernel_spmd(
            nc, in_maps, list(range(cfg["ncores"])), trace=_trace
        )
    except Exception:
        # one retry: a previously-wedged device can fail the first attempt
        res = run_bass_kernel_spmd(
            nc, in_maps, list(range(cfg["ncores"])), trace=_trace
        )
    shard = cfg["shard"]
    full = np.empty((cfg["npad"], D), dtype=np.float32)
    for c in range(cfg["ncores"]):
        full[c * shard : (c + 1) * shard] = (
            res.results[c]["outt"].astype(np.float32).T
        )
    out = full[:n]
    if _trace:
        return out, res
    return out
